# revision 97
# baseline (speedup 1.0000x reference)
"""Trainium2 Bass kernel for a 2-layer GCN encoder with global mean pool.

Sharding: dst-partition of nodes across 8 NeuronCores (12500 nodes/core,
padded to 12800 slots = 100 blocks of 128, with a cap-driven packing that
balances every (dst-block x source-window) cell to <=512 edges on all cores,
so the gather stream is only ~2.4% padded). Both convs share ONE edge-stream
layout: the conv1 gather table is raw x (bf16) permuted into slot order, and
conv1's writer emits h1*dinv(dst) in the same slot order, so conv2 gathers
with the identical indices/one-hot metadata and needs no per-edge values
(conv1 folds 1/sqrt(deg_src) into its one-hots; both convs apply the dst
scale post-GEMM). Self-loops never ride the gather stream: they are injected
straight from SBUF (raw x shard for conv1 via a dinv-scaled identity matmul,
the resident h1 accumulator for conv2 via a plain identity matmul).

Messages/one-hots/weights are bf16 (PSUM accumulates fp32): bf16 keeps the
DVE one-hot builds in the 2x perf mode, the PE at 1 cycle/row for 128-wide
outputs, and halves the h1 AllGather. Per 2-block gather group, four
dma_gather calls (one per 25600-row source window) feed TensorE accumulation
agg[feat, dst] += msg^T @ onehot(dl); the GEMM with the layer weight, bias,
dst scale and ReLU follow per 128-dst sub-block. h1 is written out per group
(overlapped) and AllGather-ed between the convs. The shard cuts sit at graph
boundaries (batch is sorted), so global_mean_pool is entirely core-local: the
one-hot-matmul pooling sums feed one fused mu/logvar head GEMM per core with
the mean divide folded per graph row, each core writes the output rows for
its own ~32 graphs, and the host assembles the final [256, 128] output from
the per-core slices (no pooling collective at all).

The host only prepares integer index/layout metadata (packing, int16 gather
indices, degrees, counts) plus dtype staging; all floating-point math
(1/sqrt scales, convs, pooling, heads) runs on device.
"""
import sys

sys.path.insert(0, "/opt/trn_rl_repo")

import numpy as np
from ml_dtypes import bfloat16

N = 100000
E = 1600000
G = 256
NCORES = 8
NSHARD = N // NCORES            # 12500 real nodes per core
NPAD = 12800                    # 100 * 128
NBLK = 100                      # dst blocks of 128 per core
BLK = 128
CH = 4                          # source windows (int16 gather idx limit)
WSZ = 25600                     # rows per source window (= 2 shards)
NSUB = 100                      # 128-row sub-blocks per core (= NBLK)
F = 128
FO = 64
SBG = 2                         # blocks per gather group
# gather groups: 12 of 8 blocks + 1 of 4
GGROUPS = [list(range(g * SBG, min((g + 1) * SBG, NBLK - 2)))
           for g in range((NBLK - 2 + SBG - 1) // SBG)]
# single-block final groups: shortens the end-of-conv drain chains
GGROUPS += [[NBLK - 2], [NBLK - 1]]
# per-cell load caps: uniform 512 packs (4 tiles per cell, ~2.4% slack)
CAP = np.full((NBLK, CH), 512, np.int64)
# vh one-hot build engine pattern (DVE=0, Pool=1); Act has no tensor ops.
# DVE is ~2.7x faster per op (bf16 2x mode) and has headroom: all-DVE.
VH_PATTERN = [0]

_CACHE = {}


def _pack_core(cnt4):
    """Assign this core's NSHARD nodes to NBLK blocks of <=128 so that the
    per-(block,chunk) cell loads stay within CAP. Greedy best-fit by peak
    cap-utilization, then a swap/move repair pass on overflowing cells."""
    loads = np.zeros((NBLK, CH), np.int64)
    counts = np.zeros(NBLK, np.int64)
    order = np.argsort(-cnt4.sum(1), kind="stable")
    block_of = np.empty(cnt4.shape[0], np.int64)
    for n in order:
        cand = loads + cnt4[n]
        util = (cand / CAP).max(1) + 1e-4 * (cand.sum(1) / CAP.sum(1))
        util[counts >= BLK] = 1e9
        b = int(np.argmin(util))
        block_of[n] = b
        loads[b] += cnt4[n]
        counts[b] += 1

    # repair: move nodes out of overflowing cells into blocks with room
    for _ in range(4000):
        over = loads - CAP
        mx = over.max()
        if mx <= 0:
            break
        b, k = np.unravel_index(np.argmax(over), over.shape)
        members = np.where(block_of == b)[0]
        msort = members[np.argsort(-cnt4[members, k])]
        moved = False
        for n in msort[:6]:
            vn = cnt4[n]
            cand = loads + vn[None, :]
            ok = ((cand <= CAP).all(1) & (counts < BLK))
            ok[b] = False
            if ok.any():
                slack = (CAP - cand).min(1).astype(np.float64)
                slack[~ok] = -1e18
                b2 = int(np.argmax(slack))
                block_of[n] = b2
                loads[b] -= vn
                loads[b2] += vn
                counts[b] -= 1
                counts[b2] += 1
                moved = True
                break
        if not moved:
            # swap with a low-load node elsewhere
            n = msort[0]
            vn = cnt4[n]
            best = None
            for b2 in np.argsort((loads / CAP).max(1))[:12]:
                if b2 == b:
                    continue
                mem2 = np.where(block_of == b2)[0]
                if len(mem2) == 0:
                    continue
                v2 = cnt4[mem2]
                nb = loads[b] - vn[None, :] + v2
                nb2 = loads[b2] + vn[None, :] - v2
                s = np.maximum((nb / CAP[b]).max(1), (nb2 / CAP[b2]).max(1))
                j = int(np.argmin(s))
                if best is None or s[j] < best[0]:
                    best = (s[j], mem2[j], b2)
            if best is None or best[0] >= (loads[b] / CAP[b]).max():
                break
            _, n2, b2 = best
            block_of[n], block_of[n2] = b2, b
            loads[b] += cnt4[n2] - vn
            loads[b2] += vn - cnt4[n2]
    return block_of


def _host_prep(x, edge_index, batch):
    srcF = edge_index[0].astype(np.int64)
    dstF = edge_index[1].astype(np.int64)
    deg = np.bincount(dstF, minlength=N).astype(np.int64) + 1
    dinv = 1.0 / np.sqrt(deg.astype(np.float64))

    # ---- shard cuts at graph boundaries ------------------------------------
    # batch is sorted; snapping each cut to the start of the graph containing
    # position c*NSHARD keeps every graph on one core, so pooling needs no
    # collective at all. Shard sizes stay within NPAD (graphs are ~390 nodes).
    batch64 = np.asarray(batch, np.int64)
    cuts = [0]
    for c in range(1, NCORES):
        cut = int(np.searchsorted(batch64, batch64[c * NSHARD]))
        while cut - cuts[-1] > NPAD:        # clamp oversized shard (rare)
            cut = int(np.searchsorted(batch64, batch64[cut - 1]))
        cuts.append(cut)
    cuts.append(N)
    owner = np.searchsorted(np.asarray(cuts), np.arange(N), side="right") - 1
    owner_e = owner[dstF]
    # source chunk of an edge = src owner pair (window of 2 shards)
    chunk_e = owner[srcF] // 2

    # ---- pack every core's dsts into blocks --------------------------------
    # (self-loops are injected on-device from SBUF via identity matmuls, so
    # they do not appear in the gather stream or the cell loads)
    block_of_g = np.empty(N, np.int64)
    slot_of_g = np.empty(N, np.int64)
    for c in range(NCORES):
        base, end = cuts[c], cuts[c + 1]
        sz = end - base
        m = owner_e == c
        ed = dstF[m] - base
        cnt4 = np.bincount(ed * CH + chunk_e[m],
                           minlength=sz * CH).reshape(sz, CH)
        blk = _pack_core(cnt4)
        block_of_g[base:end] = blk
        o = np.argsort(blk, kind="stable")
        r = np.empty(sz, np.int64)
        r[o] = np.arange(sz) - np.searchsorted(blk[o], blk[o])
        slot_of_g[base:end] = r

    # slot-major table rows: each SBUF partition (slot) owns one contiguous
    # DRAM run per shard, so the h1 writeback uses full-rate 512B+ descriptors
    tablerow = owner * NPAD + slot_of_g * NBLK + block_of_g

    # ---- per-core per-cell loads, static tile counts -----------------------
    alls, alld, allc = srcF, dstF, chunk_e
    all_owner = owner_e
    cell = block_of_g[alld] * CH + allc      # cell within owner core

    loads = np.zeros((NCORES, NBLK * CH), np.int64)
    np.add.at(loads, (all_owner, cell), 1)
    NUM = loads.max(0).reshape(NBLK, CH)              # max load per cell
    T = -(-NUM // 128)                                # tiles per cell
    T = np.maximum(T, 1)

    # stream offsets: cells ordered (ggroup, k, block-in-group)
    OFF = np.zeros((NBLK, CH), np.int64)
    GOFF = np.zeros((len(GGROUPS), CH), np.int64)     # gather offsets
    GT = np.zeros((len(GGROUPS), CH), np.int64)       # tiles per gather
    pos = 0
    for gi, blocks in enumerate(GGROUPS):
        for k in range(CH):
            GOFF[gi, k] = pos
            for b in blocks:
                OFF[b, k] = pos
                pos += T[b, k] * 128
            GT[gi, k] = (pos - GOFF[gi, k]) // 128
    NSLOT = pos
    NTILES = NSLOT // 128

    # ---- per-core edge streams --------------------------------------------
    # cuts sit at graph boundaries, so graphs are core-local: core c owns the
    # covering id range [GLO[c], GLO[c+1]-1] (empty ids inside a range still
    # produce mu = bias on device). No pooling collective is needed.
    GLO = [int(batch64[cuts[c]]) for c in range(NCORES)]
    GHI = [GLO[c + 1] - 1 for c in range(NCORES - 1)] + [G - 1]
    GSLOT = max(h - l + 1 for l, h in zip(GLO, GHI))

    dstslot = slot_of_g                                # slot within block
    per_core = []
    for c in range(NCORES):
        m = all_owner == c
        es, edd, ec = alls[m], alld[m], allc[m]
        ecell = block_of_g[edd] * CH + ec
        o = np.argsort(ecell, kind="stable")
        cell_s = ecell[o]
        cnt = np.bincount(cell_s, minlength=NBLK * CH)
        starts = np.zeros(NBLK * CH, np.int64)
        starts[1:] = np.cumsum(cnt)[:-1]
        rank = np.arange(len(cell_s)) - starts[cell_s]
        p = OFF.reshape(-1)[cell_s] + rank

        idxv = np.zeros(NSLOT, np.int16)
        dlv = np.full(NSLOT, -1.0, np.float32)
        dgv = np.ones(NSLOT, np.float32)
        idxv[p] = (tablerow[es[o]] - WSZ * ec[o]).astype(np.int16)
        dlv[p] = dstslot[edd[o]].astype(np.float32)
        dgv[p] = deg[es[o]].astype(np.float32)

        core = {}
        wrapped = np.ascontiguousarray(idxv.reshape(-1, 16).T)   # [16, NSLOT/16]
        core["idx"] = np.tile(wrapped, (8, 1))                   # [128, NSLOT/16]
        # dl in {-1,0..127} and dg (small integer degrees) are bf16-exact;
        # uploaded bf16 and widened to fp32 on device (is_equal needs fp32)
        core["dl"] = np.ascontiguousarray(
            dlv.reshape(-1, 128).T).astype(bfloat16)             # [128, NTILES]
        core["dg"] = np.ascontiguousarray(
            dgv.reshape(-1, 128).T).astype(bfloat16)             # [128, NTILES]

        nodes = np.arange(cuts[c], cuts[c + 1])
        slotidx = block_of_g[nodes] * BLK + slot_of_g[nodes]
        dv = np.ones(NPAD, np.float32)
        dv[slotidx] = deg[nodes].astype(np.float32)
        blv = np.full(NPAD, -1.0, np.float32)
        blv[slotidx] = (batch64[nodes] - GLO[c]).astype(np.float32)
        core["degd"] = np.ascontiguousarray(dv.reshape(NSUB, 128).T)
        core["bl"] = np.ascontiguousarray(blv.reshape(NSUB, 128).T)
        # per-local-graph node counts (for the on-device mean divide)
        cl = np.ones(128, np.float32)
        w = GHI[c] - GLO[c] + 1
        cl[:w] = np.bincount(batch64[nodes] - GLO[c],
                             minlength=w).astype(np.float32)
        core["cntc"] = cl.reshape(128, 1)
        per_core.append(core)

    # ---- replicated x table in slot order (bf16; scaling is on-device) -----
    xt = np.zeros((NCORES * NPAD, F), np.float32)
    xt[tablerow] = x
    x_perm_full = xt.astype(bfloat16)

    # per-core own shard in [slot, block*128+f] layout (self-loop input);
    # with slot-major table rows this is just a reshape
    for c in range(NCORES):
        sh = xt[c * NPAD:(c + 1) * NPAD]          # [NPAD, F] fp32, slot-major
        per_core[c]["xown"] = np.ascontiguousarray(
            sh.reshape(128, NSUB * F)
        ).astype(bfloat16)

    meta = dict(NSLOT=NSLOT, NTILES=NTILES, T=T, OFF=OFF, GOFF=GOFF, GT=GT,
                GSLOT=GSLOT, GLO=GLO, GHI=GHI)
    return per_core, x_perm_full, meta


def _build_bass(meta):
    from concourse import bacc, tile
    import concourse.mybir as mybir

    F32 = mybir.dt.float32
    BF16 = mybir.dt.bfloat16
    I16 = mybir.dt.int16
    EQ = mybir.AluOpType.is_equal
    MULT = mybir.AluOpType.mult
    ADD = mybir.AluOpType.add
    MAX = mybir.AluOpType.max

    NSLOT = meta["NSLOT"]
    NTILES = meta["NTILES"]
    T, OFF, GOFF, GT = meta["T"], meta["OFF"], meta["GOFF"], meta["GT"]
    GSLOT, GLO, GHI = meta["GSLOT"], meta["GLO"], meta["GHI"]

    nc = bacc.Bacc("TRN2", target_bir_lowering=False, debug=False,
                   num_devices=NCORES)

    x_tab = nc.dram_tensor("x_perm_full", [NCORES * NPAD, F], BF16,
                           kind="ExternalInput")
    xown_d = nc.dram_tensor("xown", [128, NSUB * F], BF16,
                            kind="ExternalInput")
    ident_d = nc.dram_tensor("ident", [128, 128], BF16, kind="ExternalInput")
    idx_d = nc.dram_tensor("idx", [128, NSLOT // 16], I16, kind="ExternalInput")
    dl_d = nc.dram_tensor("dl", [128, NTILES], BF16, kind="ExternalInput")
    dg_d = nc.dram_tensor("dg", [128, NTILES], BF16, kind="ExternalInput")
    iota_d = nc.dram_tensor("iota", [128, 256], BF16, kind="ExternalInput")
    degd_d = nc.dram_tensor("degd", [128, NSUB], F32, kind="ExternalInput")
    bl_d = nc.dram_tensor("bl", [128, NSUB], F32, kind="ExternalInput")
    w_d = [nc.dram_tensor(f"w{i+1}", [F, F], BF16, kind="ExternalInput")
           for i in range(2)]
    bbc_d = [nc.dram_tensor(f"b{i+1}bc", [128, F], F32, kind="ExternalInput")
             for i in range(2)]
    wcat_d = nc.dram_tensor("wcat", [F, 2 * FO], F32, kind="ExternalInput")
    bcat_d = nc.dram_tensor("bcatbc", [128, 2 * FO], F32,
                            kind="ExternalInput")
    cnt_d = nc.dram_tensor("cntc", [128, 1], F32, kind="ExternalInput")

    mulv_o = nc.dram_tensor("mulv", [128, 2 * FO], F32,
                            kind="ExternalOutput")

    with tile.TileContext(nc) as tc:
        with (
            tc.tile_pool(name="const", bufs=1) as cp,
            tc.tile_pool(name="stream", bufs=10) as sp,
            tc.tile_pool(name="work", bufs=6) as wp,
            tc.tile_pool(name="vh", bufs=56) as vp,
            tc.tile_pool(name="idxp", bufs=1) as ip,
            tc.tile_pool(name="psum", bufs=3, space="PSUM") as pp,
            tc.tile_pool(name="psum3", bufs=4, space="PSUM") as pp3,
            tc.tile_pool(name="psum1", bufs=1, space="PSUM") as pp1,
            tc.tile_pool(name="dram", bufs=1, space="DRAM") as dp,
        ):
            # ---- constants (stream metadata first: gates the first gather) -
            idxfull = ip.tile([128, NSLOT // 16], I16, tag="idxfull")
            nc.sync.dma_start(idxfull[:], idx_d[:])
            dl16 = cp.tile([128, NTILES], BF16, tag="dl16")
            nc.sync.dma_start(dl16[:], dl_d[:])
            dl_sb = cp.tile([128, NTILES], F32, tag="dl")
            nc.vector.tensor_copy(dl_sb[:], dl16[:])
            # per-edge v = rsqrt(max(deg_src, 1)) for conv1's one-hots
            dg16 = cp.tile([128, NTILES], BF16, tag="dg16")
            nc.sync.dma_start(dg16[:], dg_d[:])
            v_sb = cp.tile([128, NTILES], F32, tag="v")
            nc.vector.tensor_copy(v_sb[:], dg16[:])
            nc.vector.tensor_scalar(v_sb[:], v_sb[:], 1.0, None, MAX)
            nc.scalar.activation(v_sb[:], v_sb[:],
                                 mybir.ActivationFunctionType.Sqrt)
            nc.vector.reciprocal(v_sb[:], v_sb[:])
            iota = cp.tile([128, 256], BF16, tag="iota")
            nc.sync.dma_start(iota[:], iota_d[:])
            zeros = cp.tile([128, 512], BF16, tag="zeros")
            nc.vector.memset(zeros[:], 0.0)
            w_sb = [cp.tile([F, F], BF16, tag=f"w{i}", name=f"w{i}")
                    for i in range(2)]
            bbc_sb = [cp.tile([128, F], F32, tag=f"bbc{i}", name=f"bbc{i}")
                      for i in range(2)]
            for i in range(2):
                nc.sync.dma_start(w_sb[i][:], w_d[i][:])
                nc.sync.dma_start(bbc_sb[i][:], bbc_d[i][:])
            wcat = cp.tile([F, 2 * FO], F32, tag="wcat")
            bcat = cp.tile([128, 2 * FO], F32, tag="bcat")
            rcnt = cp.tile([128, 1], F32, tag="cnt")
            for t, d in [(wcat, wcat_d), (bcat, bcat_d), (rcnt, cnt_d)]:
                nc.sync.dma_start(t[:], d[:])
            # rcnt = 1 / max(cnt, 1), in place (per-graph, partition-major)
            nc.vector.tensor_scalar(rcnt[:], rcnt[:], 1.0, None, MAX)
            nc.vector.reciprocal(rcnt[:], rcnt[:])
            # dinvd = rsqrt(max(deg, 1)) over the dst shard, in place
            dinvd = cp.tile([128, NSUB], F32, tag="dinvd")
            nc.sync.dma_start(dinvd[:], degd_d[:])
            nc.vector.tensor_scalar(dinvd[:], dinvd[:], 1.0, None, MAX)
            nc.scalar.activation(dinvd[:], dinvd[:],
                                 mybir.ActivationFunctionType.Sqrt)
            nc.vector.reciprocal(dinvd[:], dinvd[:])
            bl_sb = cp.tile([128, NSUB], F32, tag="bl")
            nc.sync.dma_start(bl_sb[:], bl_d[:])
            ident = cp.tile([128, 128], BF16, tag="ident")
            nc.sync.dma_start(ident[:], ident_d[:])
            xown = cp.tile([128, NSUB * F], BF16, tag="xown")
            nc.sync.dma_start(xown[:], xown_d[:])

            h1acc = cp.tile([128, NSUB * 128], BF16, tag="h1acc")

            # ---- DRAM intermediates ---------------------------------------
            h1_shard = dp.tile([NPAD, F], BF16)
            h1_full = dp.tile([NPAD * NCORES, F], BF16)

            pool_ps = pp1.tile([128, GSLOT], F32, tag="pool", name="pool_ps")
            vh_count = [0]
            VENG = [nc.vector, nc.gpsimd]

            def run_conv(conv, table, selfacc, writer, group_done=None,
                         vs=None, self_diag=False):
                for gi, blocks in enumerate(GGROUPS):
                    msgs = []
                    for k in range(CH):
                        gt = int(GT[gi, k])
                        msg = sp.tile([128, gt, F], BF16, tag="msg")
                        off = int(GOFF[gi, k])
                        clen = gt * 128
                        nc.gpsimd.dma_gather(
                            msg[:, :gt, :],
                            table[WSZ * k:, :],
                            idxfull[:, off // 16: (off + clen) // 16],
                            clen, clen, F, elem_step=F,
                            single_packet=False,
                        )
                        msgs.append(msg.rearrange("p t f -> p (t f)"))
                    # PSUM sub-groups of 4 blocks
                    for s0 in range(0, len(blocks), 4):
                        sb = blocks[s0:s0 + 4]
                        agg = pp3.tile([128, 512], F32, tag="agg")
                        nc.tensor.matmul(agg[:], zeros[:, :128], zeros[:],
                                         start=True, stop=False)
                        for k in range(CH):
                            for bi, b in enumerate(sb):
                                tb = int(T[b, k])
                                base_t = (int(OFF[b, k]) - int(GOFF[gi, k])) // 128
                                for t in range(tb):
                                    col = int(OFF[b, k]) // 128 + t
                                    tl = base_t + t
                                    vh = vp.tile([128, 128], BF16, tag="vh")
                                    eng = VENG[VH_PATTERN[vh_count[0]
                                                          % len(VH_PATTERN)]]
                                    vh_count[0] += 1
                                    eng.tensor_scalar(
                                        vh[:], iota[:, :128],
                                        dl_sb[:, col:col + 1],
                                        vs[:, col:col + 1] if vs is not None
                                        else None,
                                        EQ,
                                        *( (MULT,) if vs is not None else () ),
                                    )
                                    nc.tensor.matmul(
                                        agg[:, bi * 128:(bi + 1) * 128],
                                        msgs[k][:, tl * 128:(tl + 1) * 128],
                                        vh[:],
                                        start=False, stop=False,
                                    )
                        # self-loop term: agg[:, bi] += selfacc_block^T @ D
                        # (D = identity, row-scaled by dinv when the table is
                        # unscaled raw x)
                        for bi, b in enumerate(sb):
                            if self_diag:
                                dia = vp.tile([128, 128], BF16, tag="dia")
                                nc.vector.tensor_scalar(
                                    dia[:], ident[:],
                                    dinvd[:, b:b + 1], None, MULT,
                                )
                            else:
                                dia = ident
                            nc.tensor.matmul(
                                agg[:, bi * 128:(bi + 1) * 128],
                                selfacc[:, b * F:(b + 1) * F],
                                dia[:],
                                start=False, stop=(bi == len(sb) - 1),
                            )
                        w = len(sb) * 128
                        aggT = wp.tile([128, 512], BF16, tag="aggT")
                        nc.scalar.copy(aggT[:, :w], agg[:, :w])
                        for bi, b in enumerate(sb):
                            gm = pp.tile([128, F], F32, tag="gemm")
                            nc.tensor.matmul(
                                gm[:], aggT[:, bi * 128:(bi + 1) * 128],
                                w_sb[conv][:], start=True, stop=True,
                            )
                            writer(b, gm)
                        if group_done is not None:
                            group_done(sb)

            def w_conv1(b, gm):
                h = wp.tile([128, F], F32, tag="h1w")
                nc.vector.scalar_tensor_tensor(
                    h[:], gm[:], dinvd[:, b:b + 1], bbc_sb[0][:], MULT, ADD,
                )
                nc.vector.tensor_scalar(
                    h1acc[:, b * 128:(b + 1) * 128], h[:],
                    0.0, dinvd[:, b:b + 1], MAX, MULT,
                )

            def w_conv2(b, gm):
                h2 = wp.tile([128, F], F32, tag="h2w")
                nc.vector.scalar_tensor_tensor(
                    h2[:], gm[:], dinvd[:, b:b + 1], bbc_sb[1][:], MULT, ADD,
                )
                h2r = wp.tile([128, F], BF16, tag="h2r")
                nc.vector.tensor_scalar(h2r[:], h2[:], 0.0, None, MAX)
                ph = wp.tile([128, GSLOT], BF16, tag="ph")
                nc.vector.tensor_scalar(
                    ph[:], iota[:, :GSLOT], bl_sb[:, b:b + 1], None, EQ,
                )
                nc.tensor.matmul(
                    pool_ps[:], h2r[:], ph[:],
                    start=(b == 0), stop=(b == NSUB - 1),
                )

            # write h1 out per 4-block group as it completes (overlaps conv1)
            # slot-major h1_shard: partition s holds rows [s*NBLK, (s+1)*NBLK)
            # contiguously, so writes move in full-rate 512B+ descriptors
            h1v = h1_shard.rearrange("(s b) f -> s (b f)", s=128)

            def g_conv1(sb):
                s0, n = sb[0], len(sb)
                nc.sync.dma_start(h1v[:, s0 * F:(s0 + n) * F],
                                  h1acc[:, s0 * F:(s0 + n) * F])

            run_conv(0, x_tab, xown, w_conv1, g_conv1, vs=v_sb,
                     self_diag=True)

            nc.gpsimd.collective_compute(
                "AllGather", mybir.AluOpType.bypass,
                replica_groups=[list(range(NCORES))],
                ins=[h1_shard.opt()], outs=[h1_full.opt()],
            )

            run_conv(1, h1_full, h1acc, w_conv2)

            # ---- pooling is core-local (graph-aligned shard cuts): one
            # head GEMM over this core's graph slots, no collective ----------
            pool_sb = wp.tile([128, GSLOT], F32, tag="poolsb")
            nc.vector.tensor_copy(pool_sb[:], pool_ps[:])
            hp = pp.tile([GSLOT, 2 * FO], F32, tag="gemm")
            nc.tensor.matmul(
                hp[:], pool_sb[:, :GSLOT], wcat[:], start=True, stop=True,
            )
            # mean-pool divide folded per graph row: rcnt*(sums@W) + b
            hs = wp.tile([GSLOT, 2 * FO], F32, tag="headsb")
            nc.vector.scalar_tensor_tensor(
                hs[:], hp[:], rcnt[:GSLOT, :], bcat[:GSLOT, :], MULT, ADD,
            )
            nc.sync.dma_start(mulv_o[:GSLOT, :], hs[:])

    nc.compile()
    return nc


def kernel(x, edge_index, batch, W1, b1, W2, b2, W_mu, b_mu, W_lv, b_lv):
    from concourse import bass_utils

    x = np.asarray(x, dtype=np.float32)
    edge_index = np.asarray(edge_index)
    batch = np.asarray(batch)

    per_core, x_perm_full, meta = _host_prep(x, edge_index, batch)

    iota = np.broadcast_to(np.arange(256, dtype=np.float32),
                           (128, 256)).astype(bfloat16)
    shared = dict(
        x_perm_full=x_perm_full,
        ident=np.eye(128, dtype=bfloat16),
        iota=np.ascontiguousarray(iota),
        w1=np.asarray(W1, np.float32).astype(bfloat16),
        w2=np.asarray(W2, np.float32).astype(bfloat16),
        b1bc=np.broadcast_to(np.asarray(b1, np.float32), (128, F)).copy(),
        b2bc=np.broadcast_to(np.asarray(b2, np.float32), (128, F)).copy(),
        wcat=np.concatenate([np.asarray(W_mu, np.float32),
                             np.asarray(W_lv, np.float32)], axis=1),
        bcatbc=np.broadcast_to(
            np.concatenate([np.asarray(b_mu, np.float32),
                            np.asarray(b_lv, np.float32)]), (128, 2 * FO)
        ).copy(),
    )
    in_maps = [dict(shared, **pc) for pc in per_core]

    if "nc" not in _CACHE:
        _CACHE["nc"] = _build_bass(meta)
    nc = _CACHE["nc"]

    res = bass_utils.run_bass_kernel_spmd(
        nc, in_maps, core_ids=list(range(NCORES)),
    )
    _CACHE["last_res"] = res
    # each core computed mu/logvar rows for its own (covering) graph range;
    # assemble the full [G, 2*FO] output from the per-core slices
    out = np.empty((G, 2 * FO), np.float32)
    for c in range(NCORES):
        w = meta["GHI"][c] - meta["GLO"][c] + 1
        out[meta["GLO"][c]:meta["GLO"][c] + w] = \
            res.results[c]["mulv"][:w]
    return (out[:, :FO].copy(), out[:, FO:].copy())
